# revision 5
# baseline (speedup 1.0000x reference)
"""Trainium2 Bass kernel for the MAMGCN encoder block.

Strategy: data-parallel over batch B=16 across 8 NeuronCores (2 batches/core).
Host-side prep (untimed): shard x, repack small weights, pre-transpose x to
(t*64+f, n) layout, cast matmul operands to bf16. Device does everything else:
spatial attention (two fused weight matmuls -> product -> tanh-sigmoid ->
Vs@P -> exp -> column softmax), Chebyshev graph conv with Theta folded in
(Y = X @ Theta2 block-diag), all matmuls in bf16 with fp32 PSUM accumulation.

v2: T is processed in 3 groups of 8 (each conv matmul streams exactly one
512-row PSUM bank), Y-build is interleaved with the attention phases to keep
the tensor engine dense (HAM stays at full clock), y tiles double-buffered
so Y-build overlaps the graph conv, all PSUM tiles are single-bank so the
8 banks pipeline, outputs stored as bf16.
"""
import numpy as np
import ml_dtypes

B, N, F, T, K, FO = 16, 1024, 64, 24, 3, 64
NCORES = 8
BPC = B // NCORES          # batches per core
NCH = N // 128             # 8 partition chunks of N
NJ = (T * F) // 128        # 12 chunks of the tf dim
NG = 3                     # t-groups
TT = T // NG               # 8 t's per group
NJG = NJ // NG             # 4 tf-chunks per t-group
bf16 = ml_dtypes.bfloat16

_CACHE = {}


def _build_nc():
    import concourse.bacc as bacc
    import concourse.bass as bass
    import concourse.tile as tile
    import concourse.mybir as mybir

    fp32 = mybir.dt.float32
    bf = mybir.dt.bfloat16
    AF = mybir.ActivationFunctionType

    nc = bacc.Bacc(
        "TRN2", target_bir_lowering=False, debug=False,
        enable_asserts=True, num_devices=NCORES,
    )

    # ---- DRAM I/O ----
    x_d = nc.dram_tensor("x_tf", [BPC, NJ, 128, N], bf, kind="ExternalInput")
    bs_d = nc.dram_tensor("bs_t", [NCH, 128, N], bf, kind="ExternalInput")
    vs_d = nc.dram_tensor("vs_t", [NCH, 128, N], bf, kind="ExternalInput")
    cheb_d = nc.dram_tensor("cheb_t", [K, NCH, 128, N], bf, kind="ExternalInput")
    wcat_d = nc.dram_tensor("wcat", [NJ, 128, 2 * T], bf, kind="ExternalInput")
    th2_d = nc.dram_tensor("th2", [128, 2 * K * FO], bf, kind="ExternalInput")
    hrow_d = nc.dram_tensor("hrow", [NCH, 128, 1], fp32, kind="ExternalInput")
    # out[b, group, mchunk, p, o, tl]
    out_d = nc.dram_tensor("out", [BPC, NG, NCH, 128, FO, TT], bf,
                           kind="ExternalOutput")

    with tile.TileContext(nc) as tc:
        with (
            tc.tile_pool(name="const", bufs=1) as cpool,
            tc.tile_pool(name="work", bufs=2) as wpool,
            tc.tile_pool(name="big", bufs=1) as bpool,
            tc.tile_pool(name="ypool", bufs=2) as ypool,
            tc.tile_pool(name="psA", bufs=2, space="PSUM") as psA,
            tc.tile_pool(name="psY", bufs=2, space="PSUM") as psY,
            tc.tile_pool(name="psO", bufs=2, space="PSUM") as psO,
        ):
            # ---- constants (wcat/th2 first: attention + Y-build start early)
            wcat_sb = cpool.tile([128, NJ, 2 * T], bf, tag="wcat")
            th2_sb = cpool.tile([128, 2 * K * FO], bf, tag="th2")
            vsT_sb = cpool.tile([128, NCH, N], bf, tag="vsT")
            bs_sb = cpool.tile([128, NCH, N], bf, tag="bs")
            hrow_sb = cpool.tile([128, NCH], fp32, tag="hrow")
            ones_sb = cpool.tile([128, 1], bf, tag="ones")
            one1_sb = cpool.tile([1, 1], fp32, tag="one1")
            for j in range(NJ):
                nc.sync.dma_start(wcat_sb[:, j, :], wcat_d[j])
            nc.sync.dma_start(th2_sb[:], th2_d[:])
            nc.gpsimd.memset(ones_sb[:], 1.0)
            nc.gpsimd.memset(one1_sb[:], 1.0)
            for c in range(NCH):
                nc.sync.dma_start(bs_sb[:, c, :], bs_d[c])
            for c in range(NCH):
                nc.sync.dma_start(vsT_sb[:, c, :], vs_d[c])
                nc.sync.dma_start(hrow_sb[:, c:c + 1], hrow_d[c])

            # ---- per-batch state (single-buffered big tiles) ----
            for b in range(BPC):
                x_sb = bpool.tile([128, NJ, N], bf, tag="x")
                e_sb = bpool.tile([128, NCH, N], bf, tag="e")
                p_sb = bpool.tile([128, NCH, N], bf, tag="p")
                a_sb = bpool.tile([128, K, NCH, N], bf, tag="a")
                rT_sb = bpool.tile([128, NCH], fp32, tag="rT")

                for j0 in range(0, NJ, 4):
                    nc.sync.dma_start(
                        x_sb[:, j0:j0 + 4, :],
                        x_d[b, j0:j0 + 4].rearrange("j p n -> p j n"))

                ys = []

                def build_y(g):
                    y_sb = ypool.tile([128, NCH, K, TT, FO], bf, tag="y")
                    ys.append(y_sb)
                    for j in range(NJG * g, NJG * (g + 1)):
                        tl0 = 2 * (j - NJG * g)
                        for cn in range(NCH):
                            py = psY.tile([128, 2, K, FO], fp32, tag="py")
                            nc.tensor.matmul(
                                py[:, :, :, :],
                                x_sb[:, j, cn * 128:(cn + 1) * 128],
                                th2_sb[:],
                                start=True, stop=True,
                            )
                            nc.vector.tensor_copy(
                                y_sb[:, cn, :, tl0, :], py[:, 0, :, :])
                            nc.vector.tensor_copy(
                                y_sb[:, cn, :, tl0 + 1, :], py[:, 1, :, :])

                # ---- attention pre-reductions (one pass over x) ----
                att_c = wpool.tile([2 * T, N], bf, tag="attc", bufs=1)
                att_r = wpool.tile([T, N], bf, tag="attr", bufs=1)
                pas = [psA.tile([2 * T, 512], fp32, tag="big", name=f"pa{s}")
                       for s in range(2)]
                for j in range(NJ):
                    for s in range(2):
                        nc.tensor.matmul(
                            pas[s][:, :],
                            wcat_sb[:, j, :],
                            x_sb[:, j, s * 512:(s + 1) * 512],
                            start=(j == 0), stop=(j == NJ - 1),
                        )
                for s in range(2):
                    nc.scalar.copy(att_c[:, s * 512:(s + 1) * 512], pas[s][:])
                # shift rows 24..47 down to partitions 0..23 for the product
                nc.sync.dma_start(att_r[:], att_c[T:2 * T, :])
                att_l = att_c

                # Y-build g=0 keeps the PE dense while attention drains
                build_y(0)

                # ---- product + bs -> tanh(0.5*) -> P ----
                for cn in range(NCH):
                    for s in range(2):
                        pp = psA.tile([128, 512], fp32, tag="big")
                        nc.tensor.matmul(
                            pp[:, :],
                            att_l[0:T, cn * 128:(cn + 1) * 128],
                            att_r[:, s * 512:(s + 1) * 512],
                            start=True, stop=True,
                        )
                        tmp = wpool.tile([128, 512], bf, tag="tmp")
                        nc.vector.tensor_add(
                            tmp[:], pp[:],
                            bs_sb[:, cn, s * 512:(s + 1) * 512])
                        nc.scalar.activation(
                            p_sb[:, cn, s * 512:(s + 1) * 512], tmp[:],
                            AF.Tanh, scale=0.5)

                build_y(1)

                # ---- S_pre = Vs @ P (per i-chunk), exp -> E; colsum accum;
                #      A[k, ic] = cheb * E right after each exp ----
                pcs = [psA.tile([1, 512], fp32, tag="csum", bufs=2,
                                name=f"pc{s}")
                       for s in range(2)]
                for ic in range(NCH):
                    for s in range(2):
                        ps = psA.tile([128, 512], fp32, tag="big")
                        for kc in range(NCH):
                            nc.tensor.matmul(
                                ps[:, :],
                                vsT_sb[:, kc, ic * 128:(ic + 1) * 128],
                                p_sb[:, kc, s * 512:(s + 1) * 512],
                                start=(kc == 0), stop=(kc == NCH - 1),
                            )
                        nc.scalar.activation(
                            e_sb[:, ic, s * 512:(s + 1) * 512], ps[:], AF.Exp,
                            scale=0.5, bias=hrow_sb[:, ic:ic + 1],
                        )
                        nc.tensor.matmul(
                            pcs[s][:, :],
                            ones_sb[:],
                            e_sb[:, ic, s * 512:(s + 1) * 512],
                            start=(ic == 0), stop=(ic == NCH - 1),
                        )
                    for k in range(K):
                        ch = wpool.tile([128, N], bf, tag="cheb", bufs=3)
                        nc.sync.dma_start(ch[:], cheb_d[k, ic])
                        nc.vector.tensor_mul(a_sb[:, k, ic, :], ch[:],
                                             e_sb[:, ic, :])

                # ---- column sums -> recip -> rT (128, 8) ----
                csum_sb = wpool.tile([1, N], fp32, tag="csum_s", bufs=1)
                for s in range(2):
                    nc.scalar.copy(csum_sb[:, s * 512:(s + 1) * 512],
                                   pcs[s][:])
                prt = psA.tile([128, NCH], fp32, tag="csum", bufs=2)
                for c in range(NCH):
                    nc.tensor.matmul(
                        prt[:, c:c + 1],
                        csum_sb[:, c * 128:(c + 1) * 128],
                        one1_sb[:],
                        start=True, stop=True,
                    )
                nc.vector.reciprocal(rT_sb[:], prt[:])

                # ---- graph conv per t-group (y double-buffered: build_y(2)
                #      overlaps conv g=1) ----
                def conv(g):
                    y_sb = ys[g]
                    for mc in range(NCH):
                        po = psO.tile([128, TT, FO], fp32, tag="po")
                        nmm = 0
                        for k in range(K):
                            for cn in range(NCH):
                                nc.tensor.matmul(
                                    po[:, :, :],
                                    a_sb[:, k, cn, mc * 128:(mc + 1) * 128],
                                    y_sb[:, cn, k, :, :],
                                    start=(nmm == 0), stop=(nmm == K * NCH - 1),
                                )
                                nmm += 1
                        st = wpool.tile([128, FO, TT], bf, tag="stage")
                        nc.scalar.activation(
                            st[:],
                            po[:, :, :].rearrange("p t o -> p o t"),
                            AF.Relu,
                            scale=rT_sb[:, mc:mc + 1],
                        )
                        nc.sync.dma_start(out_d[b, g, mc], st[:])

                conv(0)
                build_y(2)
                conv(1)
                conv(2)

    nc.compile()
    return nc


def _host_prep(x, W1, W2, W3, bs, Vs, cheb, Theta):
    x = np.asarray(x, np.float32)
    W1 = np.asarray(W1, np.float32)
    W2 = np.asarray(W2, np.float32)
    W3 = np.asarray(W3, np.float32)
    bs = np.asarray(bs, np.float32)
    Vs = np.asarray(Vs, np.float32)
    cheb = np.asarray(cheb, np.float32)
    Theta = np.asarray(Theta, np.float32)

    x_tf = np.ascontiguousarray(x.transpose(0, 3, 2, 1)).reshape(B, NJ, 128, N)
    x_tf = x_tf.astype(bf16)
    bs_t = bs[0].reshape(NCH, 128, N).astype(bf16)
    vs_t = np.ascontiguousarray(Vs.T).reshape(NCH, 128, N).astype(bf16)
    cheb_t = cheb.reshape(K, NCH, 128, N).astype(bf16)
    t_idx = np.arange(T * F) // F
    f_idx = np.arange(T * F) % F
    wl_flat = W1[t_idx][:, None] * W2[f_idx, :]
    wr_flat = np.zeros((T * F, T), np.float32)
    wr_flat[np.arange(T * F), t_idx] = W3[f_idx]
    wcat = np.concatenate([wl_flat, wr_flat], axis=1)
    wcat = wcat.reshape(NJ, 128, 2 * T).astype(bf16)
    th2 = np.zeros((128, 2 * K * FO), np.float32)
    for par in range(2):
        for k in range(K):
            th2[par * F:(par + 1) * F,
                par * K * FO + k * FO:(par * K + k + 1) * FO] = Theta[k]
    th2 = th2.astype(bf16)
    hrow = (0.5 * Vs.sum(axis=1)).astype(np.float32).reshape(NCH, 128, 1)
    return x_tf, bs_t, vs_t, cheb_t, wcat, th2, hrow


def kernel(x, W1, W2, W3, bs, Vs, cheb, Theta, _return_results=False,
           _trace=False):
    from concourse.bass_utils import run_bass_kernel_spmd

    x_tf, bs_t, vs_t, cheb_t, wcat, th2, hrow = _host_prep(
        x, W1, W2, W3, bs, Vs, cheb, Theta)

    if "nc" not in _CACHE:
        _CACHE["nc"] = _build_nc()
    nc = _CACHE["nc"]

    shared = dict(bs_t=bs_t, vs_t=vs_t, cheb_t=cheb_t, wcat=wcat,
                  th2=th2, hrow=hrow)
    in_maps = []
    for c in range(NCORES):
        m = dict(shared)
        m["x_tf"] = np.ascontiguousarray(x_tf[c * BPC:(c + 1) * BPC])
        in_maps.append(m)

    _CACHE["in_maps"] = in_maps
    kw = {"trace": True} if _trace else {}
    res = run_bass_kernel_spmd(nc, in_maps, list(range(NCORES)), **kw)
    outs = []
    for c in range(NCORES):
        o = res.results[c]["out"]  # (BPC, NG, NCH, 128, FO, TT)
        o = np.asarray(o, np.float32)
        o = o.transpose(0, 2, 3, 4, 1, 5).reshape(BPC, N, FO, T)
        outs.append(o)
    full = np.concatenate(outs, axis=0).astype(np.float32)
    if _return_results:
        return full, res
    return full
